# revision 1
# baseline (speedup 1.0000x reference)
"""Trainium2 Bass kernel for nn_Aggregate (2D rel-pos attention, 2 fmaps).

Math (per fmap, per batch, per head):
  q = SCALE * (Wq @ fmap)                      # (128, HW)  d x i, i=(x,y) H-major
  hs(x,y,u) = q(:,x,y) . rel_h[x-u+99]         # H-direction rel-pos logits
  ws(x,y,v) = q(:,x,y) . rel_w[y-v+99]         # W-direction rel-pos logits
  S(i, j=(u,v)) = hs + ws ; A = softmax_j(S)
  out = A @ V ; proj = gamma * Wp_h @ out

Key restructuring for TRN2:
  exp(hs+ws) = exp(hs) * exp(ws)  -- exp only on small factors (Eht, Ewt)
  softmax division deferred:  A@V = (E@V) / den,  den = (sum_u e^hs)(sum_v e^ws)
  E^T built chunk-by-chunk in (j-part, i-free) layout:
     E^T_c = EwtD  *  broadcast(Eht rows 2c, 2c+1)
  broadcast via DMA free-step-0 APs / gpsimd.partition_broadcast,
  multiply on DVE bf16 2x mode, attn@V on PE with K=128 chunks,
  denominators via ones-vector matmuls, division done on host (linearity).

Sharding: 16 head-instances = 2 fmaps x 2 batch x 4 heads -> 8 cores,
2 heads per core (same fmap/batch slice). Host sums the per-head
projection contributions and adds the residual.
"""
import numpy as np
import ml_dtypes
from contextlib import ExitStack

import concourse.bass as bass
import concourse.tile as tile
import concourse.mybir as mybir
from concourse import bacc, bass_utils
from concourse.bass_types import AP

F32 = mybir.dt.float32
BF16 = mybir.dt.bfloat16

HEADS = 4
DH = 128
DIM = 128
MAX_POS = 100
SCALE = DH ** -0.5
B = 2
H = 48
W = 64
HW = H * W          # 3072
NCHUNK = HW // 128  # 24
NBLK = HW // 512    # 6

# chunks whose EhtB broadcast runs on GPSIMD instead of DMA. Disabled: the
# partition_broadcast ucode needs partition-0 sources (staging added too much
# critical-path latency to pay off in the end-to-end schedule).
GPS_CHUNKS = set()  # gpsimd partition_broadcast produces NaN on real HW; all-DMA broadcast

_cached = {}


def _build_nc():
    if "nc" in _cached:
        return _cached["nc"]
    nc = bacc.Bacc("TRN2", target_bir_lowering=False, debug=False)

    fmapb_d = nc.dram_tensor("fmapb", [128, HW], BF16, kind="ExternalInput").ap()
    wqt_d = nc.dram_tensor("wqt", [128, 256], BF16, kind="ExternalInput").ap()
    wvt_d = nc.dram_tensor("wvt", [128, 256], BF16, kind="ExternalInput").ap()
    wpt_d = nc.dram_tensor("wpt", [128, 256], BF16, kind="ExternalInput").ap()
    het_d = nc.dram_tensor("het", [128, H * H], BF16, kind="ExternalInput").ap()
    wet_d = nc.dram_tensor("wet", [128, W * W], BF16, kind="ExternalInput").ap()
    out_d = [nc.dram_tensor(f"out{h}", [128, HW], F32, kind="ExternalOutput").ap()
             for h in range(2)]
    den_d = nc.dram_tensor("den", [4, HW], BF16, kind="ExternalOutput").ap()

    with tile.TileContext(nc) as tc, ExitStack() as ctx:
        pool = ctx.enter_context(tc.tile_pool(name="sb", bufs=1))

        # ---- load inputs ----
        fmapb = pool.tile([128, HW], BF16)
        nc.sync.dma_start(fmapb[:], fmapb_d[:])
        wqt = pool.tile([128, 256], BF16)
        nc.sync.dma_start(wqt[:], wqt_d[:])
        wvt = pool.tile([128, 256], BF16)
        nc.sync.dma_start(wvt[:], wvt_d[:])
        wpt = pool.tile([128, 256], BF16)
        nc.sync.dma_start(wpt[:], wpt_d[:])
        het = pool.tile([128, H * H], BF16)
        nc.sync.dma_start(het[:], het_d[:])
        wet = pool.tile([128, W * W], BF16)
        nc.sync.dma_start(wet[:], wet_d[:])
        ones48 = pool.tile([48, 1], BF16)
        nc.vector.memset(ones48[:], 1.0)
        ones64 = pool.tile([64, 1], BF16)
        nc.vector.memset(ones64[:], 1.0)

        v2 = pool.tile([128, NCHUNK * 256], BF16)  # (j_in_chunk, c*256 + h*128 + d)
        q2h = [pool.tile([128, HW], BF16, name=f"q2h{h}") for h in range(2)]
        ehth = [pool.tile([48, HW], BF16, name=f"ehth{h}") for h in range(2)]
        ewtdh = [pool.tile([128, HW], BF16, name=f"ewtdh{h}") for h in range(2)]
        q2vh = [q2h[h][:, :].rearrange("p (x y) -> p x y", x=H, y=W) for h in range(2)]

        gps_stage = {}
        ps = ctx.enter_context(tc.tile_pool(name="ps", bufs=2, space="PSUM"))
        ebpool = ctx.enter_context(tc.tile_pool(name="eb", bufs=6))
        etpool = ctx.enter_context(tc.tile_pool(name="et", bufs=4))
        nmpool = ctx.enter_context(tc.tile_pool(name="nm", bufs=2))

        def prep_head(h, pp):
            # q
            for b in range(NBLK):
                qp = pp.tile([128, 512], F32, tag="ps", name=f"qp{h}{b}")
                nc.tensor.matmul(qp[:], wqt[:, h * 128:(h + 1) * 128],
                                 fmapb[:, b * 512:(b + 1) * 512],
                                 start=True, stop=True)
                nc.vector.tensor_copy(
                    q2h[h][:, b * 512:(b + 1) * 512], qp[:])
            # hs^T -> exp(eht); groups of 8 x, contiguous dst
            for xg in range(H // 8):
                hsp = pp.tile([48, 512], F32, tag="ps", name=f"hsp{h}{xg}")
                for xi in range(8):
                    x = xg * 8 + xi
                    nc.tensor.matmul(hsp[:, xi * W:(xi + 1) * W],
                                     het[:, x * 48:(x + 1) * 48],
                                     q2vh[h][:, x, :], start=True, stop=True)
                nc.scalar.activation(
                    ehth[h][:, xg * 512:(xg + 1) * 512], hsp[:],
                    mybir.ActivationFunctionType.Exp)
            # ws^T -> exp(ewtd rows 0..63); groups of 8 y, strided dst
            for yg in range(W // 8):
                wsp = pp.tile([64, 384], F32, tag="ps", name=f"wsp{h}{yg}")
                for yi in range(8):
                    y = yg * 8 + yi
                    nc.tensor.matmul(wsp[:, yi * 48:(yi + 1) * 48],
                                     wet[:, y * 64:(y + 1) * 64],
                                     q2vh[h][:, :, y], start=True, stop=True)
                ssl = wsp[:, :]
                srcap = AP(ssl.tensor, ssl.offset, [ssl.ap[0], [48, 8], [1, 48]])
                dsl = ewtdh[h][0:64, yg * 8: yg * 8 + 1]
                dst = AP(dsl.tensor, dsl.offset, [dsl.ap[0], [1, 8], [W, 48]])
                nc.scalar.activation(dst, srcap, mybir.ActivationFunctionType.Exp)
            # duplicate Ewt rows into partitions 64..127
            nc.sync.dma_start(ewtdh[h][64:128, :], ewtdh[h][0:64, :])
            # pre-stage GPS chunks' row pairs at 32-aligned partitions
            # (partition_broadcast requires 32-aligned source partitions)
            for c in range(NCHUNK):
                if (h, c) in GPS_CHUNKS:
                    stgs = []
                    for du in range(2):
                        stg = ebpool.tile([1, HW], BF16, tag="gstage",
                                          name=f"stg{h}{c}{du}", bufs=4)
                        nc.sync.dma_start(
                            stg[:], ehth[h][2 * c + du: 2 * c + du + 1, :])
                        stgs.append(stg)
                    gps_stage[(h, c)] = stgs

        def chunks_head(h, psO):
            outp = [psO.tile([128, 512], F32, tag="po", name=f"outp_h{h}_{b}")
                    for b in range(NBLK)]
            for c in range(NCHUNK):
                ehtb = ebpool.tile([128, HW], BF16, tag="eb", name=f"ehtb{h}{c}")
                for du in range(2):
                    srcrow = ehth[h][2 * c + du: 2 * c + du + 1, :]
                    if (h, c) in GPS_CHUNKS:
                        nc.gpsimd.partition_broadcast(
                            ehtb[du * 64:(du + 1) * 64, :],
                            gps_stage[(h, c)][du][:])
                    else:
                        bsrc = AP(srcrow.tensor, srcrow.offset,
                                  [srcrow.ap[0], [0, 64], [1, HW]])
                        nc.sync.dma_start(ehtb[du * 64:(du + 1) * 64, :], bsrc)
                et = etpool.tile([128, HW], BF16, tag="et", name=f"et{h}{c}")
                half = HW // 2
                nc.vector.tensor_mul(et[:, 0:half],
                                     ewtdh[h][:, 0:half], ehtb[:, 0:half])
                nc.vector.tensor_mul(et[:, half:HW],
                                     ewtdh[h][:, half:HW], ehtb[:, half:HW])
                for b in range(NBLK):
                    nc.tensor.matmul(outp[b][:],
                                     v2[:, c * 256 + h * 128: c * 256 + (h + 1) * 128],
                                     et[:, b * 512:(b + 1) * 512],
                                     start=(c == 0), stop=(c == NCHUNK - 1))
            # numerator -> sbuf bf16 (ACT is idle during chunk phase)
            numh = nmpool.tile([128, HW], BF16, tag="nm", name=f"numh{h}")
            for b in range(NBLK):
                nc.scalar.copy(numh[:, b * 512:(b + 1) * 512], outp[b][:])
            return numh

        def proj_head(h, numh):
            for b in range(NBLK):
                pp = ps.tile([128, 512], F32, tag="ps", name=f"pp{h}{b}")
                nc.tensor.matmul(pp[:], wpt[:, h * 128:(h + 1) * 128],
                                 numh[:, b * 512:(b + 1) * 512],
                                 start=True, stop=True)
                po = nmpool.tile([128, 512], F32, tag="po", name=f"po{h}{b}")
                nc.scalar.copy(po[:], pp[:])
                nc.sync.dma_start(out_d[h][:, b * 512:(b + 1) * 512], po[:])

        def dens(h, kind, psD):
            dp = psD.tile([1, HW], F32, tag="pd", name=f"dp{h}{kind}")
            for b in range(NBLK):
                if kind == 0:
                    nc.tensor.matmul(dp[:, b * 512:(b + 1) * 512], ones48[:],
                                     ehth[h][:, b * 512:(b + 1) * 512],
                                     start=True, stop=True)
                else:
                    nc.tensor.matmul(dp[:, b * 512:(b + 1) * 512], ones64[:],
                                     ewtdh[h][0:64, b * 512:(b + 1) * 512],
                                     start=True, stop=True)
            dsb = nmpool.tile([1, HW], BF16, tag="dsb", name=f"densb{h}{kind}")
            nc.vector.tensor_copy(dsb[:], dp[:])
            nc.sync.dma_start(den_d[2 * h + kind: 2 * h + kind + 1, :], dsb[:])

        psPrep_cm = tc.tile_pool(name="psPrep", bufs=6, space="PSUM")
        psPrep = psPrep_cm.__enter__()
        # V in (j, d) layout, both heads (needs only fmapb)
        for c in range(NCHUNK):
            vp = psPrep.tile([128, 256], F32, tag="ps", name=f"vp{c}")
            nc.tensor.matmul(vp[:], fmapb[:, c * 128:(c + 1) * 128], wvt[:],
                             start=True, stop=True)
            nc.scalar.copy(v2[:, c * 256:(c + 1) * 256], vp[:])

        prep_head(0, psPrep)
        psPrep_cm.__exit__(None, None, None)
        with tc.tile_pool(name="psD0", bufs=1, space="PSUM") as psD0:
            dens(0, 0, psD0)
            dens(0, 1, psD0)
        with tc.tile_pool(name="psO0", bufs=6, space="PSUM") as psO0:
            prep_head(1, ps)
            numh0 = chunks_head(0, psO0)
        with tc.tile_pool(name="psD1", bufs=1, space="PSUM") as psD1:
            dens(1, 0, psD1)
            dens(1, 1, psD1)
        with tc.tile_pool(name="psO1", bufs=6, space="PSUM") as psO1:
            proj_head(0, numh0)
            numh1 = chunks_head(1, psO1)
        proj_head(1, numh1)

    nc.compile()
    _cached["nc"] = nc
    return nc


def _prep_core_inputs(fmap_cb, Wqk, Wv, rel_h, rel_w, Wp, gamma, pair):
    """Host-side input prep for one core. fmap_cb: (128, HW) f32 slice."""
    bf = ml_dtypes.bfloat16
    hg0 = pair * 2  # global head index of local head 0
    wqt = np.empty((128, 256), np.float32)
    wvt = np.empty((128, 256), np.float32)
    wpt = np.empty((128, 256), np.float32)
    for hl in range(2):
        hg = hg0 + hl
        wqt[:, hl * 128:(hl + 1) * 128] = SCALE * Wqk[hg * 128:(hg + 1) * 128, :].T
        wvt[:, hl * 128:(hl + 1) * 128] = Wv[hg * 128:(hg + 1) * 128, :].T
        # wpt[d, hl*128 + c] = gamma * Wp[c, hg*128 + d]
        wpt[:, hl * 128:(hl + 1) * 128] = gamma * Wp[:, hg * 128:(hg + 1) * 128].T
    idx_h = np.arange(H)[:, None] - np.arange(H)[None, :] + (MAX_POS - 1)
    idx_w = np.arange(W)[:, None] - np.arange(W)[None, :] + (MAX_POS - 1)
    het = rel_h[idx_h].transpose(2, 0, 1).reshape(128, H * H)  # (d, x*48+u)
    wet = rel_w[idx_w].transpose(2, 0, 1).reshape(128, W * W)  # (d, y*64+v)
    return {
        "fmapb": fmap_cb.astype(bf),
        "wqt": wqt.astype(bf),
        "wvt": wvt.astype(bf),
        "wpt": wpt.astype(bf),
        "het": het.astype(bf),
        "wet": wet.astype(bf),
    }


def kernel(fmap1, fmap2, Wqk, Wv, rel_h, rel_w, Wp, gamma):
    fmap1 = np.asarray(fmap1, np.float32)
    fmap2 = np.asarray(fmap2, np.float32)
    Wqk = np.asarray(Wqk, np.float32)
    Wv = np.asarray(Wv, np.float32)
    rel_h = np.asarray(rel_h, np.float32)
    rel_w = np.asarray(rel_w, np.float32)
    Wp = np.asarray(Wp, np.float32)
    g = float(np.asarray(gamma).reshape(-1)[0])

    nc = _build_nc()
    fmaps = [fmap1, fmap2]
    in_maps = []
    core_meta = []
    for pair in range(2):
        for f in range(2):
            for b in range(B):
                fm = fmaps[f][b].reshape(DIM, HW)
                in_maps.append(_prep_core_inputs(fm, Wqk, Wv, rel_h, rel_w, Wp, g, pair))
                core_meta.append((pair, f, b))

    res = bass_utils.run_bass_kernel_spmd(nc, in_maps, core_ids=list(range(8)))

    outs = [np.array(fmaps[f], np.float32).copy() for f in range(2)]
    for core, (pair, f, b) in enumerate(core_meta):
        r = res.results[core]
        den = np.asarray(r["den"], np.float32)
        for hl in range(2):
            num = r[f"out{hl}"]                       # (128, HW) gamma-scaled numerator
            d = den[2 * hl] * den[2 * hl + 1]          # (HW,)
            outs[f][b] += (num / d[None, :]).reshape(DIM, H, W)
    return outs[0], outs[1]



# revision 28
# speedup vs baseline: 1.5476x; 1.5476x over previous
"""Trainium2 Bass kernel for nn_Aggregate (2D rel-pos attention, 2 fmaps).

Math (per fmap, per batch, per head):
  q = SCALE * (Wq @ fmap)                      # (128, HW)  d x i
  hs(x,y,u) = q(:,x,y) . rel_h[x-u+99]         # H-direction rel-pos logits
  ws(x,y,v) = q(:,x,y) . rel_w[y-v+99]         # W-direction rel-pos logits
  S(i, j=(u,v)) = hs + ws ; A = softmax_j(S)
  out = A @ V ; proj = gamma * Wp_h @ out

v4 structure:
  - exp(hs+ws) = exp(hs)*exp(ws): exp only on the small factors.
  - q is never materialized: the host folds SCALE*Wq_h^T into the rel-pos
    tables (het2 = SCALE*Wq_h^T@het, wet2 likewise), so hs/ws logits are
    single matmuls against fmap -- the exps are ready ~6us into the kernel.
  - Key chunks are (8u x 16v) blocks, row j = ul*16+vl.  The host
    pre-permutes fmap columns into blocked spatial order
      pos(x,y) = 512*(x//8) + 128*(y//16) + 16*(x%8) + (y%16)
    so contiguous 128-col V-matmul chunks ARE the key chunks; query columns
    inherit the order and the host un-permutes outputs.
  - E^T chunk (b,w) = EWREP_w * EHREP_b, built from 10 rep-tiles/head:
    EHREP_b by one u-major replication DMA each; EWREP_w by a 4-DMA
    partition-doubling chain (16->32->64->128 rows).
  - The elementwise multiplies run on DVE (bf16 2x) with ~4/24 chunks
    offloaded to GpSimd at pipelined slots; attn numerator on PE with
    K=128 chunks into 6 PSUM banks; PSUM->SBUF copies on ACT.
  - Denominator sums, the division, and the Wp projection happen on the
    host (linearity: proj(num)/den == proj(num/den)).

Sharding: 16 head-instances = 2 fmaps x 2 batch x 4 heads -> 8 cores,
2 heads per core (same fmap/batch slice).
"""
import numpy as np
import ml_dtypes
from contextlib import ExitStack

import concourse.bass as bass
import concourse.tile as tile
import concourse.mybir as mybir
from concourse import bacc, bass_utils
from concourse.bass_types import AP

F32 = mybir.dt.float32
BF16 = mybir.dt.bfloat16

HEADS = 4
DH = 128
DIM = 128
MAX_POS = 100
SCALE = DH ** -0.5
B = 2
H = 48
W = 64
HW = H * W            # 3072
UB = 6                # u-blocks (8 u's each)
WB = 4                # v-blocks (16 v's each)
NCHUNK = UB * WB      # 24 key chunks of 128
NBLK = HW // 512      # 6 query blocks
F8 = mybir.dt.float8e4

# Chunk-stream order (per head): chunks ordered by rep-tile arrival
# (DMA issue order: [w0 chain] EH0 EH1 [w1 chain] EH2 [w2] [w3] EH3..EH5).
# GpSimd computes the w2 column (early-arriving tiles, ~6us/multiply of
# lead time); it is consumed at the stream tail.  Head 1's rep tiles are
# all ready during head 0's stream, so GpSimd takes its whole w2 column.
POOL_CHUNKS0 = [(0, 2), (1, 2), (2, 2), (3, 2)]
DVE_CHUNKS0 = [(0, 0), (1, 0), (0, 1), (1, 1), (2, 0), (2, 1),
               (0, 3), (1, 3), (2, 3), (3, 0), (3, 1), (3, 3),
               (4, 0), (4, 1), (4, 3), (5, 0), (5, 1), (5, 3),
               (4, 2), (5, 2)]
POOL_CHUNKS1 = POOL_CHUNKS0
DVE_CHUNKS1 = DVE_CHUNKS0

_cached = {}


def _perm():
    # pos[x*64+y] = device column index of spatial (x, y)
    x = np.arange(H)[:, None]
    y = np.arange(W)[None, :]
    pos = 512 * (x // 8) + 128 * (y // 16) + 16 * (x % 8) + (y % 16)
    return pos.ravel()


def _build_nc():
    if "nc" in _cached:
        return _cached["nc"]
    nc = bacc.Bacc("TRN2", target_bir_lowering=False, debug=False)

    fmapb_d = nc.dram_tensor("fmapb", [128, HW], F8, kind="ExternalInput").ap()
    wvt_d = nc.dram_tensor("wvt", [128, 256], BF16, kind="ExternalInput").ap()
    het2_d = nc.dram_tensor("het2", [128, 2 * H * H], F8, kind="ExternalInput").ap()
    wet2_d = nc.dram_tensor("wet2", [128, 2 * W * W], F8, kind="ExternalInput").ap()
    num_d = [nc.dram_tensor(f"num{h}", [128, HW], BF16, kind="ExternalOutput").ap()
             for h in range(2)]
    eh_d = [nc.dram_tensor(f"eh{h}", [H, HW], BF16, kind="ExternalOutput").ap()
            for h in range(2)]
    ew_d = [nc.dram_tensor(f"ew{h}", [W, HW], BF16, kind="ExternalOutput").ap()
            for h in range(2)]

    with tile.TileContext(nc) as tc, ExitStack() as ctx:
        pool = ctx.enter_context(tc.tile_pool(name="sb", bufs=1))

        # ---- load inputs (head-0 halves first so prep can start early) ----
        fmapb = pool.tile([128, HW], F8)
        nc.sync.dma_start(fmapb[:], fmapb_d[:])
        wet2 = pool.tile([128, 2 * W * W], F8)
        nc.sync.dma_start(wet2[:, 0:4096], wet2_d[:, 0:4096])
        het2 = pool.tile([128, 2 * H * H], F8)
        nc.sync.dma_start(het2[:, 0:2304], het2_d[:, 0:2304])
        wvt = pool.tile([128, 256], BF16)
        nc.sync.dma_start(wvt[:], wvt_d[:])
        nc.sync.dma_start(wet2[:, 4096:8192], wet2_d[:, 4096:8192])
        nc.sync.dma_start(het2[:, 2304:4608], het2_d[:, 2304:4608])

        v2 = pool.tile([128, NCHUNK * 256], BF16)  # (j_in_chunk, c*256 + h*128 + d)
        ehth = [pool.tile([H, HW], BF16, name=f"ehth{h}") for h in range(2)]
        ewth = [pool.tile([W, HW], BF16, name=f"ewth{h}") for h in range(2)]

        ps = ctx.enter_context(tc.tile_pool(name="ps", bufs=2, space="PSUM"))
        psO = ctx.enter_context(tc.tile_pool(name="psO", bufs=6, space="PSUM"))
        ehr = ctx.enter_context(tc.tile_pool(name="ehr", bufs=8))
        ewr = ctx.enter_context(tc.tile_pool(name="ewr", bufs=7))
        etd = ctx.enter_context(tc.tile_pool(name="etd", bufs=4))
        etg = ctx.enter_context(tc.tile_pool(name="etg", bufs=4))
        nmp = ctx.enter_context(tc.tile_pool(name="nmp", bufs=1))

        def prep_head(h):
            fm = fmapb[:, :]
            # ws first: the EWREP chains are the long DMA pole.
            # query y: w_q = y//16, vl_q = y%16;
            # fmap cols for fixed y: 512b + 128*w_q + 16*ul + vl_q
            for yg in range(W // 8):
                wsp = ps.tile([64, 384], F32, tag="ps", name=f"wsp{h}{yg}")
                for yi in range(8):
                    y = yg * 8 + yi
                    rhs = AP(fm.tensor, fm.offset + 128 * (y // 16) + (y % 16),
                             [fm.ap[0], [512, 6], [16, 8]])
                    nc.tensor.matmul(wsp[:, yi * 48:(yi + 1) * 48],
                                     wet2[:, h * 4096 + y * 64:h * 4096 + (y + 1) * 64],
                                     rhs, start=True, stop=True)
                # exp: src (yi, b, ul); dst ewt[v, 512b+16ul+128*(yg//2)+8*(yg%2)+yi]
                ssl = wsp[:, :]
                srcap = AP(ssl.tensor, ssl.offset, [ssl.ap[0], [48, 8], [8, 6], [1, 8]])
                dsl = ewth[h][:, :]
                dst = AP(dsl.tensor, dsl.offset + 128 * (yg // 2) + 8 * (yg % 2),
                         [dsl.ap[0], [1, 8], [512, 6], [16, 8]])
                nc.scalar.activation(dst, srcap, mybir.ActivationFunctionType.Exp)
            # hs: x = 8*xg+ul; fmap cols for fixed x: 512*xg+16ul + 128w + vl
            for xg in range(H // 8):
                hsp = ps.tile([48, 512], F32, tag="ps", name=f"hsp{h}{xg}")
                for ul in range(8):
                    x = xg * 8 + ul
                    rhs = AP(fm.tensor, fm.offset + 512 * xg + 16 * ul,
                             [fm.ap[0], [128, 4], [1, 16]])
                    nc.tensor.matmul(hsp[:, ul * 64:(ul + 1) * 64],
                                     het2[:, h * 2304 + x * 48:h * 2304 + (x + 1) * 48],
                                     rhs, start=True, stop=True)
                # exp: src (ul, w, vl); dst eht[u, 512*xg + 16ul + 128w + vl]
                ssl = hsp[:, :]
                srcap = AP(ssl.tensor, ssl.offset, [ssl.ap[0], [64, 8], [16, 4], [1, 16]])
                dsl = ehth[h][:, :]
                dst = AP(dsl.tensor, dsl.offset + 512 * xg,
                         [dsl.ap[0], [16, 8], [128, 4], [1, 16]])
                nc.scalar.activation(dst, srcap, mybir.ActivationFunctionType.Exp)

        def rep_tiles(h):
            # EWREP_w: row j -> ewt[16w + j%16]; 4-DMA doubling chain each.
            # EHREP_b: row j -> eht[8b + j//16]; ONE u-major replication DMA.
            ews, ehs = [], []
            def one_ew(w):
                # depth-2 replication: seed rows 0:32 with two DMAs from ewt,
                # then three independent 32-row copies off the seed (each
                # extra chain level costs a ~900ns completion-sem hop, and
                # chained links head-of-line-block the serial DMA issue
                # queue -- keep the dependency depth at 2).
                t = ewr.tile([128, HW], BF16, tag="ewr", name=f"ewr{h}{w}")
                sw = ewth[h][16 * w:16 * (w + 1), :]
                nc.sync.dma_start(t[0:16, :], sw)
                nc.sync.dma_start(t[16:32, :], sw)
                ta = t[:, :]
                src = AP(ta.tensor, ta.offset, [[HW, 32], [1, HW]])
                for base in (32, 64, 96):
                    dst = AP(ta.tensor, ta.offset + base * HW, [[HW, 32], [1, HW]])
                    nc.sync.dma_start(dst, src)
                return t
            def one_eh(b):
                t = ehr.tile([128, HW], BF16, tag="ehr", name=f"ehr{h}{b}")
                s = ehth[h][:, :]
                src = AP(s.tensor, s.offset + (8 * b) * HW,
                         [[HW, 8], [0, 16], [1, HW]])
                nc.sync.dma_start(t[:], src)
                return t
            # issue order: [w0] EH0 EH1 [w1] EH2 [w2] [w3] EH3 EH4 EH5
            ews, ehs = [None] * WB, [None] * UB
            ews[0] = one_ew(0)
            ehs[0] = one_eh(0)
            ehs[1] = one_eh(1)
            ews[1] = one_ew(1)
            ehs[2] = one_eh(2)
            ews[2] = one_ew(2)
            ews[3] = one_ew(3)
            ehs[3] = one_eh(3)
            ehs[4] = one_eh(4)
            ehs[5] = one_eh(5)
            return ews, ehs

        def chunks_head(h, ews, ehs, pool_chunks, dve_chunks):
            STREAM = dve_chunks + pool_chunks
            outp = [psO.tile([128, 512], F32, tag="po", name=f"outp{h}{blk}")
                    for blk in range(NBLK)]
            # GpSimd pre-computes the pool_chunks multiplies (consumed at the
            # stream tail; ~6us/multiply of lead time).
            pool_et = {}
            for (b, w) in pool_chunks:
                et = etg.tile([128, HW], BF16, tag="etg", name=f"etg{h}{b}{w}")
                nc.gpsimd.tensor_mul(et[:], ews[w][:, :], ehs[b][:, :])
                pool_et[(b, w)] = et
            for k, (b, w) in enumerate(STREAM):
                c = 4 * b + w  # v2 / PSUM-accumulation chunk id
                if (b, w) in pool_et:
                    et = pool_et[(b, w)]
                else:
                    et = etd.tile([128, HW], BF16, tag="etd", name=f"etd{h}{b}{w}")
                    nc.vector.tensor_mul(et[:], ews[w][:, :], ehs[b][:, :])
                for blk in range(NBLK):
                    nc.tensor.matmul(outp[blk][:],
                                     v2[:, c * 256 + h * 128: c * 256 + (h + 1) * 128],
                                     et[:, blk * 512:(blk + 1) * 512],
                                     start=(k == 0), stop=(k == NCHUNK - 1))
            numh = nmp.tile([128, HW], BF16, tag="nm", name=f"numh{h}")
            for blk in range(NBLK):
                # alternate ACT / DVE so the tail drains twice as fast
                if blk % 2 == 0:
                    nc.scalar.copy(numh[:, blk * 512:(blk + 1) * 512], outp[blk][:])
                else:
                    nc.vector.tensor_copy(numh[:, blk * 512:(blk + 1) * 512], outp[blk][:])
                if blk == 2:
                    nc.sync.dma_start(num_d[h][:, 0:1536], numh[:, 0:1536])
            nc.sync.dma_start(num_d[h][:, 1536:HW], numh[:, 1536:HW])

        # ---- schedule ----
        prep_head(0)
        rep0 = rep_tiles(0)
        # V2 for both heads; fmapb columns are pre-permuted so natural
        # 128-col blocks are the blocked key chunks.
        for c in range(NCHUNK):
            vp = ps.tile([128, 256], F32, tag="ps", name=f"vp{c}")
            nc.tensor.matmul(vp[:], fmapb[:, c * 128:(c + 1) * 128], wvt[:],
                             start=True, stop=True)
            nc.scalar.copy(v2[:, c * 256:(c + 1) * 256], vp[:])
        prep_head(1)
        rep1 = rep_tiles(1)
        nc.sync.dma_start(eh_d[0][:], ehth[0][:])
        nc.sync.dma_start(ew_d[0][:], ewth[0][:])
        chunks_head(0, *rep0, POOL_CHUNKS0, DVE_CHUNKS0)
        nc.sync.dma_start(eh_d[1][:], ehth[1][:])
        nc.sync.dma_start(ew_d[1][:], ewth[1][:])
        chunks_head(1, *rep1, POOL_CHUNKS1, DVE_CHUNKS1)

    nc.compile()
    _cached["nc"] = nc
    return nc


def _prep_core_inputs(fmap_cb, Wqk, Wv, rel_h, rel_w, pair, perm):
    """Host-side input prep for one core. fmap_cb: (128, HW) f32 slice."""
    bf = ml_dtypes.bfloat16
    hg0 = pair * 2  # global head index of local head 0
    wvt = np.empty((128, 256), np.float32)
    het2 = np.empty((128, 2 * H * H), np.float32)
    wet2 = np.empty((128, 2 * W * W), np.float32)
    idx_h = np.arange(H)[:, None] - np.arange(H)[None, :] + (MAX_POS - 1)
    idx_w = np.arange(W)[:, None] - np.arange(W)[None, :] + (MAX_POS - 1)
    het = rel_h[idx_h].transpose(2, 0, 1).reshape(128, H * H)  # (d, x*48+u)
    wet = rel_w[idx_w].transpose(2, 0, 1).reshape(128, W * W)  # (d, y*64+v)
    for hl in range(2):
        hg = hg0 + hl
        wq = Wqk[hg * 128:(hg + 1) * 128, :]          # (d, c)
        wvt[:, hl * 128:(hl + 1) * 128] = Wv[hg * 128:(hg + 1) * 128, :].T
        het2[:, hl * H * H:(hl + 1) * H * H] = SCALE * (wq.T @ het)
        wet2[:, hl * W * W:(hl + 1) * W * W] = SCALE * (wq.T @ wet)
    fperm = np.empty_like(fmap_cb)
    fperm[:, perm] = fmap_cb
    return {
        "fmapb": fperm.astype(ml_dtypes.float8_e4m3fn),
        "wvt": wvt.astype(bf),
        "het2": het2.astype(ml_dtypes.float8_e4m3fn),
        "wet2": wet2.astype(ml_dtypes.float8_e4m3fn),
    }


def kernel(fmap1, fmap2, Wqk, Wv, rel_h, rel_w, Wp, gamma):
    fmap1 = np.asarray(fmap1, np.float32)
    fmap2 = np.asarray(fmap2, np.float32)
    Wqk = np.asarray(Wqk, np.float32)
    Wv = np.asarray(Wv, np.float32)
    rel_h = np.asarray(rel_h, np.float32)
    rel_w = np.asarray(rel_w, np.float32)
    Wp = np.asarray(Wp, np.float32)
    g = float(np.asarray(gamma).reshape(-1)[0])
    perm = _perm()  # perm[x*64+y] = device column of spatial (x,y)

    nc = _build_nc()
    fmaps = [fmap1, fmap2]
    in_maps = []
    core_meta = []
    for pair in range(2):
        for f in range(2):
            for b in range(B):
                fm = fmaps[f][b].reshape(DIM, HW)
                in_maps.append(_prep_core_inputs(fm, Wqk, Wv, rel_h, rel_w,
                                                 pair, perm))
                core_meta.append((pair, f, b))

    res = bass_utils.run_bass_kernel_spmd(nc, in_maps, core_ids=list(range(8)))

    outs = [np.array(fmaps[f], np.float32).copy() for f in range(2)]
    for core, (pair, f, b) in enumerate(core_meta):
        r = res.results[core]
        for hl in range(2):
            hg = pair * 2 + hl
            num = np.asarray(r[f"num{hl}"], np.float32)       # (128, HW) permuted
            den = (np.asarray(r[f"eh{hl}"], np.float32).sum(0)
                   * np.asarray(r[f"ew{hl}"], np.float32).sum(0))  # permuted
            attn = num / den[None, :]
            attn = attn[:, perm]                              # back to spatial
            proj = g * (Wp[:, hg * 128:(hg + 1) * 128] @ attn)
            outs[f][b] += proj.reshape(DIM, H, W)
    return outs[0], outs[1]


# revision 47
# speedup vs baseline: 1.7105x; 1.1052x over previous
"""Trainium2 Bass kernel for nn_Aggregate (2D rel-pos attention, 2 fmaps).

Math (per fmap, per batch, per head):
  q = SCALE * (Wq @ fmap)                      # (128, HW)  d x i
  hs(x,y,u) = q(:,x,y) . rel_h[x-u+99]         # H-direction rel-pos logits
  ws(x,y,v) = q(:,x,y) . rel_w[y-v+99]         # W-direction rel-pos logits
  S(i, j=(u,v)) = hs + ws ; A = softmax_j(S)
  out = A @ V ; proj = gamma * Wp_h @ out

v4 structure:
  - exp(hs+ws) = exp(hs)*exp(ws): exp only on the small factors.
  - q is never materialized: the host folds SCALE*Wq_h^T into the rel-pos
    tables (het2 = SCALE*Wq_h^T@het, wet2 likewise), so hs/ws logits are
    single matmuls against fmap -- the exps are ready ~6us into the kernel.
  - Key chunks are (8u x 16v) blocks, row j = ul*16+vl.  The host
    pre-permutes fmap columns into blocked spatial order
      pos(x,y) = 512*(x//8) + 128*(y//16) + 16*(x%8) + (y%16)
    so contiguous 128-col V-matmul chunks ARE the key chunks; query columns
    inherit the order and the host un-permutes outputs.
  - E^T chunk (b,w) = EWREP_w * EHREP_b, built from 10 rep-tiles/head:
    EHREP_b by one u-major replication DMA each; EWREP_w by a 4-DMA
    partition-doubling chain (16->32->64->128 rows).
  - The elementwise multiplies run on DVE (bf16 2x) with ~4/24 chunks
    offloaded to GpSimd at pipelined slots; attn numerator on PE with
    K=128 chunks into 6 PSUM banks; PSUM->SBUF copies on ACT.
  - Denominator sums, the division, and the Wp projection happen on the
    host (linearity: proj(num)/den == proj(num/den)).

Sharding: 16 head-instances = 2 fmaps x 2 batch x 4 heads -> 8 cores,
2 heads per core (same fmap/batch slice).
"""
import numpy as np
import ml_dtypes
from contextlib import ExitStack

import concourse.bass as bass
import concourse.tile as tile
import concourse.mybir as mybir
from concourse import bacc, bass_utils
from concourse.bass_types import AP

F32 = mybir.dt.float32
BF16 = mybir.dt.bfloat16

HEADS = 4
DH = 128
DIM = 128
MAX_POS = 100
SCALE = DH ** -0.5
B = 2
H = 48
W = 64
HW = H * W            # 3072
UB = 6                # u-blocks (8 u's each)
WB = 4                # v-blocks (16 v's each)
NCHUNK = UB * WB      # 24 key chunks of 128
NBLK = HW // 512      # 6 query blocks
F8 = mybir.dt.float8e4

# Chunk-stream order (per head): chunks ordered by rep-tile arrival
# (DMA issue order: [w0 chain] EH0 EH1 [w1 chain] EH2 [w2] [w3] EH3..EH5).
# Every chunk's elementwise multiply is split DVE (cols 0:SPL) +
# GpSimd (cols SPL:), so the two engines pace evenly with PE.
STREAM = [(0, 0), (1, 0), (0, 1), (1, 1), (2, 0), (2, 1),
          (0, 2), (1, 2), (2, 2), (0, 3), (1, 3), (2, 3),
          (3, 0), (3, 1), (3, 2), (3, 3), (4, 0), (4, 1),
          (4, 2), (4, 3), (5, 0), (5, 1), (5, 2), (5, 3)]
SPL = 2432  # DVE | GpSimd column split of each chunk multiply

_cached = {}


def _perm():
    # pos[x*64+y] = device column index of spatial (x, y)
    x = np.arange(H)[:, None]
    y = np.arange(W)[None, :]
    pos = 512 * (x // 8) + 128 * (y // 16) + 16 * (x % 8) + (y % 16)
    return pos.ravel()


def _build_nc():
    if "nc" in _cached:
        return _cached["nc"]
    nc = bacc.Bacc("TRN2", target_bir_lowering=False, debug=False)

    fmapb_d = nc.dram_tensor("fmapb", [128, HW], F8, kind="ExternalInput").ap()
    wvt_d = nc.dram_tensor("wvt", [128, 256], BF16, kind="ExternalInput").ap()
    het2_d = nc.dram_tensor("het2", [128, 2 * H * H], F8, kind="ExternalInput").ap()
    wet2_d = nc.dram_tensor("wet2", [128, 2 * W * W], F8, kind="ExternalInput").ap()
    num_d = [nc.dram_tensor(f"num{h}", [128, HW], BF16, kind="ExternalOutput").ap()
             for h in range(2)]
    eh_d = [nc.dram_tensor(f"eh{h}", [H, HW], BF16, kind="ExternalOutput").ap()
            for h in range(2)]
    ew_d = [nc.dram_tensor(f"ew{h}", [W, HW], BF16, kind="ExternalOutput").ap()
            for h in range(2)]

    with tile.TileContext(nc) as tc, ExitStack() as ctx:
        pool = ctx.enter_context(tc.tile_pool(name="sb", bufs=1))

        # ---- load inputs (head-0 halves first so prep can start early) ----
        fmapb = pool.tile([128, HW], F8)
        nc.sync.dma_start(fmapb[:], fmapb_d[:])
        wet2 = pool.tile([128, 2 * W * W], F8)
        nc.sync.dma_start(wet2[:, 0:4096], wet2_d[:, 0:4096])
        het2 = pool.tile([128, 2 * H * H], F8)
        nc.sync.dma_start(het2[:, 0:2304], het2_d[:, 0:2304])
        wvt = pool.tile([128, 256], BF16)
        nc.sync.dma_start(wvt[:], wvt_d[:])
        nc.sync.dma_start(wet2[:, 4096:8192], wet2_d[:, 4096:8192])
        nc.sync.dma_start(het2[:, 2304:4608], het2_d[:, 2304:4608])

        v2 = pool.tile([128, NCHUNK * 256], BF16)  # (j_in_chunk, c*256 + h*128 + d)
        ehth = [pool.tile([H, HW], BF16, name=f"ehth{h}") for h in range(2)]
        ewth = [pool.tile([W, HW], BF16, name=f"ewth{h}") for h in range(2)]

        ps = ctx.enter_context(tc.tile_pool(name="ps", bufs=2, space="PSUM"))
        psO = ctx.enter_context(tc.tile_pool(name="psO", bufs=6, space="PSUM"))
        ehr = ctx.enter_context(tc.tile_pool(name="ehr", bufs=8))
        ewr = ctx.enter_context(tc.tile_pool(name="ewr", bufs=7))
        etd = ctx.enter_context(tc.tile_pool(name="etd", bufs=6))
        nmp = ctx.enter_context(tc.tile_pool(name="nmp", bufs=1))

        def prep_head(h):
            fm = fmapb[:, :]
            # ws first: the EWREP chains are the long DMA pole.
            # query y: w_q = y//16, vl_q = y%16;
            # fmap cols for fixed y: 512b + 128*w_q + 16*ul + vl_q
            for yg in range(W // 8):
                wsp = ps.tile([64, 384], F32, tag="ps", name=f"wsp{h}{yg}")
                for yi in range(8):
                    y = yg * 8 + yi
                    rhs = AP(fm.tensor, fm.offset + 128 * (y // 16) + (y % 16),
                             [fm.ap[0], [512, 6], [16, 8]])
                    nc.tensor.matmul(wsp[:, yi * 48:(yi + 1) * 48],
                                     wet2[:, h * 4096 + y * 64:h * 4096 + (y + 1) * 64],
                                     rhs, start=True, stop=True)
                # exp: src (yi, b, ul); dst ewt[v, 512b+16ul+128*(yg//2)+8*(yg%2)+yi]
                ssl = wsp[:, :]
                srcap = AP(ssl.tensor, ssl.offset, [ssl.ap[0], [48, 8], [8, 6], [1, 8]])
                dsl = ewth[h][:, :]
                dst = AP(dsl.tensor, dsl.offset + 128 * (yg // 2) + 8 * (yg % 2),
                         [dsl.ap[0], [1, 8], [512, 6], [16, 8]])
                nc.scalar.activation(dst, srcap, mybir.ActivationFunctionType.Exp)
            # hs: x = 8*xg+ul; fmap cols for fixed x: 512*xg+16ul + 128w + vl
            for xg in range(H // 8):
                hsp = ps.tile([48, 512], F32, tag="ps", name=f"hsp{h}{xg}")
                for ul in range(8):
                    x = xg * 8 + ul
                    rhs = AP(fm.tensor, fm.offset + 512 * xg + 16 * ul,
                             [fm.ap[0], [128, 4], [1, 16]])
                    nc.tensor.matmul(hsp[:, ul * 64:(ul + 1) * 64],
                                     het2[:, h * 2304 + x * 48:h * 2304 + (x + 1) * 48],
                                     rhs, start=True, stop=True)
                # exp: src (ul, w, vl); dst eht[u, 512*xg + 16ul + 128w + vl]
                ssl = hsp[:, :]
                srcap = AP(ssl.tensor, ssl.offset, [ssl.ap[0], [64, 8], [16, 4], [1, 16]])
                dsl = ehth[h][:, :]
                dst = AP(dsl.tensor, dsl.offset + 512 * xg,
                         [dsl.ap[0], [16, 8], [128, 4], [1, 16]])
                nc.scalar.activation(dst, srcap, mybir.ActivationFunctionType.Exp)

        def rep_tiles(h):
            # EWREP_w: row j -> ewt[16w + j%16]; depth-2 5-DMA replication.
            # EHREP_b: row j -> eht[8b + j//16]; ONE u-major replication DMA
            # (6 column-split DMAs for head 0's EHREP_0 so it streams out
            # behind the per-group hs exps).
            def one_eh(b, parts=1):
                t = ehr.tile([128, HW], BF16, tag="ehr", name=f"ehr{h}{b}")
                s = ehth[h][:, :]
                cw = HW // parts
                for p in range(parts):
                    src = AP(s.tensor, s.offset + (8 * b) * HW + p * cw,
                             [[HW, 8], [0, 16], [1, cw]])
                    nc.sync.dma_start(t[:, p * cw:(p + 1) * cw], src)
                return t
            def ew_seed(w):
                t = ewr.tile([128, HW], BF16, tag="ewr", name=f"ewr{h}{w}")
                sw = ewth[h][16 * w:16 * (w + 1), :]
                nc.sync.dma_start(t[0:16, :], sw)
                nc.sync.dma_start(t[16:32, :], sw)
                return t
            def ew_copies(t, parts=1):
                ta = t[:, :]
                cw = HW // parts
                for p in range(parts):
                    src = AP(ta.tensor, ta.offset + p * cw, [[HW, 32], [1, cw]])
                    for base in (32, 64, 96):
                        dst = AP(ta.tensor, ta.offset + base * HW + p * cw,
                                 [[HW, 32], [1, cw]])
                        nc.sync.dma_start(dst, src)
                return t
            ews, ehs = [None] * WB, [None] * UB
            if h == 0:
                # hand-woven issue order: every chained DMA's predecessor
                # sem has fired by the time the serial issue queue reaches
                # it, EHREP_0 streams behind the hs exps, and each tile
                # lands just before its first consuming chunk slot.
                s0 = ew_seed(0)
                ews[0] = ew_copies(s0)
                ehs[0] = one_eh(0, parts=6)
                ehs[1] = one_eh(1)
                s1 = ew_seed(1)
                ews[1] = ew_copies(s1)
                ehs[2] = one_eh(2)
                s2 = ew_seed(2)
                ews[2] = ew_copies(s2)
                s3 = ew_seed(3)
                ews[3] = ew_copies(s3)
                ehs[3] = one_eh(3)
                ehs[4] = one_eh(4)
                ehs[5] = one_eh(5)
            else:
                ews[0] = ew_copies(ew_seed(0))
                ehs[0] = one_eh(0)
                ehs[1] = one_eh(1)
                ews[2] = ew_copies(ew_seed(2))
                ews[1] = ew_copies(ew_seed(1))
                ehs[2] = one_eh(2)
                ews[3] = ew_copies(ew_seed(3))
                ehs[3] = one_eh(3)
                ehs[4] = one_eh(4)
                ehs[5] = one_eh(5)
            return ews, ehs

        def chunks_head(h, ews, ehs, stream):
            outp = [psO.tile([128, 512], F32, tag="po", name=f"outp{h}{blk}")
                    for blk in range(NBLK)]
            for k, (b, w) in enumerate(stream):
                c = 4 * b + w  # v2 / PSUM-accumulation chunk id
                et = etd.tile([128, HW], BF16, tag="etd", name=f"etd{h}{b}{w}")
                nc.vector.tensor_mul(et[:, 0:SPL],
                                     ews[w][:, 0:SPL], ehs[b][:, 0:SPL])
                nc.gpsimd.tensor_mul(et[:, SPL:HW],
                                     ews[w][:, SPL:HW], ehs[b][:, SPL:HW])
                for blk in range(NBLK):
                    nc.tensor.matmul(outp[blk][:],
                                     v2[:, c * 256 + h * 128: c * 256 + (h + 1) * 128],
                                     et[:, blk * 512:(blk + 1) * 512],
                                     start=(k == 0), stop=(k == NCHUNK - 1))
            # tail drain: ACT+DVE copy in parallel rounds; ship each
            # 1024-col pair as soon as both copies land.
            numh = nmp.tile([128, HW], BF16, tag="nm", name=f"numh{h}")
            for r in range(3):
                a, b_ = 2 * r, 2 * r + 1
                nc.scalar.copy(numh[:, a * 512:(a + 1) * 512], outp[a][:])
                nc.vector.tensor_copy(numh[:, b_ * 512:(b_ + 1) * 512], outp[b_][:])
                nc.sync.dma_start(num_d[h][:, a * 512:(b_ + 1) * 512],
                                  numh[:, a * 512:(b_ + 1) * 512])

        # ---- schedule ----
        prep_head(0)
        rep0 = rep_tiles(0)
        # V2 for both heads; fmapb columns are pre-permuted so natural
        # 128-col blocks are the blocked key chunks.
        for c in range(NCHUNK):
            vp = ps.tile([128, 256], F32, tag="ps", name=f"vp{c}")
            nc.tensor.matmul(vp[:], fmapb[:, c * 128:(c + 1) * 128], wvt[:],
                             start=True, stop=True)
            nc.scalar.copy(v2[:, c * 256:(c + 1) * 256], vp[:])
        prep_head(1)
        rep1 = rep_tiles(1)
        nc.sync.dma_start(eh_d[0][:], ehth[0][:])
        nc.sync.dma_start(ew_d[0][:], ewth[0][:])
        chunks_head(0, *rep0, STREAM)
        nc.sync.dma_start(eh_d[1][:], ehth[1][:])
        nc.sync.dma_start(ew_d[1][:], ewth[1][:])
        chunks_head(1, *rep1, STREAM)

    nc.compile()
    _cached["nc"] = nc
    return nc


def _prep_core_inputs(fmap_cb, Wqk, Wv, rel_h, rel_w, pair, perm):
    """Host-side input prep for one core. fmap_cb: (128, HW) f32 slice."""
    bf = ml_dtypes.bfloat16
    hg0 = pair * 2  # global head index of local head 0
    wvt = np.empty((128, 256), np.float32)
    het2 = np.empty((128, 2 * H * H), np.float32)
    wet2 = np.empty((128, 2 * W * W), np.float32)
    idx_h = np.arange(H)[:, None] - np.arange(H)[None, :] + (MAX_POS - 1)
    idx_w = np.arange(W)[:, None] - np.arange(W)[None, :] + (MAX_POS - 1)
    het = rel_h[idx_h].transpose(2, 0, 1).reshape(128, H * H)  # (d, x*48+u)
    wet = rel_w[idx_w].transpose(2, 0, 1).reshape(128, W * W)  # (d, y*64+v)
    for hl in range(2):
        hg = hg0 + hl
        wq = Wqk[hg * 128:(hg + 1) * 128, :]          # (d, c)
        wvt[:, hl * 128:(hl + 1) * 128] = Wv[hg * 128:(hg + 1) * 128, :].T
        het2[:, hl * H * H:(hl + 1) * H * H] = SCALE * (wq.T @ het)
        wet2[:, hl * W * W:(hl + 1) * W * W] = SCALE * (wq.T @ wet)
    fperm = np.empty_like(fmap_cb)
    fperm[:, perm] = fmap_cb
    return {
        "fmapb": fperm.astype(ml_dtypes.float8_e4m3fn),
        "wvt": wvt.astype(bf),
        "het2": het2.astype(ml_dtypes.float8_e4m3fn),
        "wet2": wet2.astype(ml_dtypes.float8_e4m3fn),
    }


def kernel(fmap1, fmap2, Wqk, Wv, rel_h, rel_w, Wp, gamma):
    fmap1 = np.asarray(fmap1, np.float32)
    fmap2 = np.asarray(fmap2, np.float32)
    Wqk = np.asarray(Wqk, np.float32)
    Wv = np.asarray(Wv, np.float32)
    rel_h = np.asarray(rel_h, np.float32)
    rel_w = np.asarray(rel_w, np.float32)
    Wp = np.asarray(Wp, np.float32)
    g = float(np.asarray(gamma).reshape(-1)[0])
    perm = _perm()  # perm[x*64+y] = device column of spatial (x,y)

    nc = _build_nc()
    fmaps = [fmap1, fmap2]
    in_maps = []
    core_meta = []
    for pair in range(2):
        for f in range(2):
            for b in range(B):
                fm = fmaps[f][b].reshape(DIM, HW)
                in_maps.append(_prep_core_inputs(fm, Wqk, Wv, rel_h, rel_w,
                                                 pair, perm))
                core_meta.append((pair, f, b))

    res = bass_utils.run_bass_kernel_spmd(nc, in_maps, core_ids=list(range(8)))

    outs = [np.array(fmaps[f], np.float32).copy() for f in range(2)]
    for core, (pair, f, b) in enumerate(core_meta):
        r = res.results[core]
        for hl in range(2):
            hg = pair * 2 + hl
            num = np.asarray(r[f"num{hl}"], np.float32)       # (128, HW) permuted
            den = (np.asarray(r[f"eh{hl}"], np.float32).sum(0)
                   * np.asarray(r[f"ew{hl}"], np.float32).sum(0))  # permuted
            attn = num / den[None, :]
            attn = attn[:, perm]                              # back to spatial
            proj = g * (Wp[:, hg * 128:(hg + 1) * 128] @ attn)
            outs[f][b] += proj.reshape(DIM, H, W)
    return outs[0], outs[1]


# revision 59
# speedup vs baseline: 1.7368x; 1.0154x over previous
"""Trainium2 Bass kernel for nn_Aggregate (2D rel-pos attention, 2 fmaps).

Math (per fmap, per batch, per head):
  q = SCALE * (Wq @ fmap)                      # (128, HW)  d x i
  hs(x,y,u) = q(:,x,y) . rel_h[x-u+99]         # H-direction rel-pos logits
  ws(x,y,v) = q(:,x,y) . rel_w[y-v+99]         # W-direction rel-pos logits
  S(i, j=(u,v)) = hs + ws ; A = softmax_j(S)
  out = A @ V ; proj = gamma * Wp_h @ out

v4 structure:
  - exp(hs+ws) = exp(hs)*exp(ws): exp only on the small factors.
  - q is never materialized: the host folds SCALE*Wq_h^T into the rel-pos
    tables (het2 = SCALE*Wq_h^T@het, wet2 likewise), so hs/ws logits are
    single matmuls against fmap -- the exps are ready ~6us into the kernel.
  - Key chunks are (8u x 16v) blocks, row j = ul*16+vl.  The host
    pre-permutes fmap columns into blocked spatial order
      pos(x,y) = 512*(x//8) + 128*(y//16) + 16*(x%8) + (y%16)
    so contiguous 128-col V-matmul chunks ARE the key chunks; query columns
    inherit the order and the host un-permutes outputs.
  - E^T chunk (b,w) = EWREP_w * EHREP_b, built from 10 rep-tiles/head:
    EHREP_b by one u-major replication DMA each; EWREP_w by a 4-DMA
    partition-doubling chain (16->32->64->128 rows).
  - The elementwise multiplies run on DVE (bf16 2x) with ~4/24 chunks
    offloaded to GpSimd at pipelined slots; attn numerator on PE with
    K=128 chunks into 6 PSUM banks; PSUM->SBUF copies on ACT.
  - Denominator sums, the division, and the Wp projection happen on the
    host (linearity: proj(num)/den == proj(num/den)).

Sharding: 16 head-instances = 2 fmaps x 2 batch x 4 heads -> 8 cores,
2 heads per core (same fmap/batch slice).
"""
import numpy as np
import ml_dtypes
from contextlib import ExitStack

import concourse.bass as bass
import concourse.tile as tile
import concourse.mybir as mybir
from concourse import bacc, bass_utils
from concourse.bass_types import AP

F32 = mybir.dt.float32
BF16 = mybir.dt.bfloat16

HEADS = 4
DH = 128
DIM = 128
MAX_POS = 100
SCALE = DH ** -0.5
B = 2
H = 48
W = 64
HW = H * W            # 3072
UB = 6                # u-blocks (8 u's each)
WB = 4                # v-blocks (16 v's each)
NCHUNK = UB * WB      # 24 key chunks of 128
NBLK = HW // 512      # 6 query blocks
F8 = mybir.dt.float8e4

# Chunk-stream order (per head): chunks ordered by rep-tile arrival
# (DMA issue order: [w0 chain] EH0 EH1 [w1 chain] EH2 [w2] [w3] EH3..EH5).
# Every chunk's elementwise multiply is split DVE (cols 0:SPL) +
# GpSimd (cols SPL:), so the two engines pace evenly with PE.
STREAM = [(0, 0), (1, 0), (0, 1), (1, 1), (2, 0), (2, 1),
          (0, 2), (1, 2), (2, 2), (0, 3), (1, 3), (2, 3),
          (3, 0), (3, 1), (3, 2), (3, 3), (4, 0), (4, 1),
          (4, 2), (4, 3), (5, 0), (5, 1), (5, 2), (5, 3)]
SPL = 2432  # DVE | GpSimd column split of each chunk multiply

_cached = {}


def _perm():
    # pos[x*64+y] = device column index of spatial (x, y)
    x = np.arange(H)[:, None]
    y = np.arange(W)[None, :]
    pos = 512 * (x // 8) + 128 * (y // 16) + 16 * (x % 8) + (y % 16)
    return pos.ravel()


def _build_nc():
    if "nc" in _cached:
        return _cached["nc"]
    nc = bacc.Bacc("TRN2", target_bir_lowering=False, debug=False)

    fmapb_d = nc.dram_tensor("fmapb", [128, HW], F8, kind="ExternalInput").ap()
    wvt_d = nc.dram_tensor("wvt", [128, 256], BF16, kind="ExternalInput").ap()
    het2_d = nc.dram_tensor("het2", [128, 2 * H * H], F8, kind="ExternalInput").ap()
    wet2_d = nc.dram_tensor("wet2", [128, 2 * W * W], F8, kind="ExternalInput").ap()
    num_d = [nc.dram_tensor(f"num{h}", [128, HW], BF16, kind="ExternalOutput").ap()
             for h in range(2)]
    eh_d = [nc.dram_tensor(f"eh{h}", [H, HW], BF16, kind="ExternalOutput").ap()
            for h in range(2)]
    ew_d = [nc.dram_tensor(f"ew{h}", [W, HW], BF16, kind="ExternalOutput").ap()
            for h in range(2)]

    with tile.TileContext(nc) as tc, ExitStack() as ctx:
        pool = ctx.enter_context(tc.tile_pool(name="sb", bufs=1))

        # ---- load inputs (head-0 halves first so prep can start early) ----
        fmapb = pool.tile([128, HW], F8)
        nc.sync.dma_start(fmapb[:], fmapb_d[:])
        wet2 = pool.tile([128, 2 * W * W], F8)
        nc.sync.dma_start(wet2[:, 0:4096], wet2_d[:, 0:4096])
        het2 = pool.tile([128, 2 * H * H], F8)
        nc.sync.dma_start(het2[:, 0:2304], het2_d[:, 0:2304])
        wvt = pool.tile([128, 256], BF16)
        nc.sync.dma_start(wvt[:], wvt_d[:])
        nc.sync.dma_start(wet2[:, 4096:8192], wet2_d[:, 4096:8192])
        nc.sync.dma_start(het2[:, 2304:4608], het2_d[:, 2304:4608])

        v2 = pool.tile([128, NCHUNK * 256], BF16)  # (j_in_chunk, c*256 + h*128 + d)
        ehth = [pool.tile([H, HW], BF16, name=f"ehth{h}") for h in range(2)]
        ewth = [pool.tile([W, HW], BF16, name=f"ewth{h}") for h in range(2)]

        ps = ctx.enter_context(tc.tile_pool(name="ps", bufs=2, space="PSUM"))
        psO = ctx.enter_context(tc.tile_pool(name="psO", bufs=6, space="PSUM"))
        ehr = ctx.enter_context(tc.tile_pool(name="ehr", bufs=8))
        ewr = ctx.enter_context(tc.tile_pool(name="ewr", bufs=7))
        etd = ctx.enter_context(tc.tile_pool(name="etd", bufs=8))
        nmp = ctx.enter_context(tc.tile_pool(name="nmp", bufs=1))

        def prep_head(h):
            fm = fmapb[:, :]
            # ws first: the EWREP chains are the long DMA pole.
            # query y: w_q = y//16, vl_q = y%16;
            # fmap cols for fixed y: 512b + 128*w_q + 16*ul + vl_q
            for yg in range(W // 8):
                wsp = ps.tile([64, 384], F32, tag="ps", name=f"wsp{h}{yg}")
                for yi in range(8):
                    y = yg * 8 + yi
                    rhs = AP(fm.tensor, fm.offset + 128 * (y // 16) + (y % 16),
                             [fm.ap[0], [512, 6], [16, 8]])
                    nc.tensor.matmul(wsp[:, yi * 48:(yi + 1) * 48],
                                     wet2[:, h * 4096 + y * 64:h * 4096 + (y + 1) * 64],
                                     rhs, start=True, stop=True)
                # exp: src (yi, b, ul); dst ewt[v, 512b+16ul+128*(yg//2)+8*(yg%2)+yi]
                ssl = wsp[:, :]
                srcap = AP(ssl.tensor, ssl.offset, [ssl.ap[0], [48, 8], [8, 6], [1, 8]])
                dsl = ewth[h][:, :]
                dst = AP(dsl.tensor, dsl.offset + 128 * (yg // 2) + 8 * (yg % 2),
                         [dsl.ap[0], [1, 8], [512, 6], [16, 8]])
                nc.scalar.activation(dst, srcap, mybir.ActivationFunctionType.Exp)
            # hs: x = 8*xg+ul; fmap cols for fixed x: 512*xg+16ul + 128w + vl
            for xg in range(H // 8):
                hsp = ps.tile([48, 512], F32, tag="ps", name=f"hsp{h}{xg}")
                for ul in range(8):
                    x = xg * 8 + ul
                    rhs = AP(fm.tensor, fm.offset + 512 * xg + 16 * ul,
                             [fm.ap[0], [128, 4], [1, 16]])
                    nc.tensor.matmul(hsp[:, ul * 64:(ul + 1) * 64],
                                     het2[:, h * 2304 + x * 48:h * 2304 + (x + 1) * 48],
                                     rhs, start=True, stop=True)
                # exp: src (ul, w, vl); dst eht[u, 512*xg + 16ul + 128w + vl]
                ssl = hsp[:, :]
                srcap = AP(ssl.tensor, ssl.offset, [ssl.ap[0], [64, 8], [16, 4], [1, 16]])
                dsl = ehth[h][:, :]
                dst = AP(dsl.tensor, dsl.offset + 512 * xg,
                         [dsl.ap[0], [16, 8], [128, 4], [1, 16]])
                nc.scalar.activation(dst, srcap, mybir.ActivationFunctionType.Exp)

        def rep_tiles(h):
            # EWREP_w: row j -> ewt[16w + j%16]; depth-2 5-DMA replication.
            # EHREP_b: row j -> eht[8b + j//16]; ONE u-major replication DMA
            # (6 column-split DMAs for head 0's EHREP_0 so it streams out
            # behind the per-group hs exps).
            def eh_part(t, b, p, cw):
                s = ehth[h][:, :]
                src = AP(s.tensor, s.offset + (8 * b) * HW + p * cw,
                         [[HW, 8], [0, 16], [1, cw]])
                nc.sync.dma_start(t[:, p * cw:(p + 1) * cw], src)
            def one_eh(b, parts=1):
                t = ehr.tile([128, HW], BF16, tag="ehr", name=f"ehr{h}{b}")
                for p in range(parts):
                    eh_part(t, b, p, HW // parts)
                return t
            def ew_seed(w):
                t = ewr.tile([128, HW], BF16, tag="ewr", name=f"ewr{h}{w}")
                sw = ewth[h][16 * w:16 * (w + 1), :]
                nc.sync.dma_start(t[0:16, :], sw)
                nc.sync.dma_start(t[16:32, :], sw)
                return t
            def ew_copies(t, parts=1):
                ta = t[:, :]
                cw = HW // parts
                for p in range(parts):
                    src = AP(ta.tensor, ta.offset + p * cw, [[HW, 32], [1, cw]])
                    for base in (32, 64, 96):
                        dst = AP(ta.tensor, ta.offset + base * HW + p * cw,
                                 [[HW, 32], [1, cw]])
                        nc.sync.dma_start(dst, src)
                return t
            ews, ehs = [None] * WB, [None] * UB
            if h == 0:
                # hand-woven issue order: every chained DMA's predecessor
                # sem has fired by the time the serial issue queue reaches
                # it, EHREP_0 streams behind the hs exps, and each tile
                # lands just before its first consuming chunk slot.
                s0 = ew_seed(0)
                ews[0] = ew_copies(s0)
                ehs[0] = one_eh(0, parts=6)
                ehs[1] = one_eh(1)
                s1 = ew_seed(1)
                ews[1] = ew_copies(s1)
                ehs[2] = one_eh(2)
                s2 = ew_seed(2)
                ews[2] = ew_copies(s2)
                s3 = ew_seed(3)
                ews[3] = ew_copies(s3)
                ehs[3] = one_eh(3)
                ehs[4] = one_eh(4)
                ehs[5] = one_eh(5)
            else:
                ews[0] = ew_copies(ew_seed(0))
                ehs[0] = one_eh(0)
                ehs[1] = one_eh(1)
                ews[2] = ew_copies(ew_seed(2))
                ews[1] = ew_copies(ew_seed(1))
                ehs[2] = one_eh(2)
                ews[3] = ew_copies(ew_seed(3))
                ehs[3] = one_eh(3)
                ehs[4] = one_eh(4)
                ehs[5] = one_eh(5)
            return ews, ehs

        def chunks_head(h, ews, ehs, stream):
            outp = [psO.tile([128, 512], F32, tag="po", name=f"outp{h}{blk}")
                    for blk in range(NBLK)]
            for k, (b, w) in enumerate(stream):
                c = 4 * b + w  # v2 / PSUM-accumulation chunk id
                et = etd.tile([128, HW], BF16, tag="etd", name=f"etd{h}{b}{w}")
                nc.vector.tensor_mul(et[:, 0:SPL],
                                     ews[w][:, 0:SPL], ehs[b][:, 0:SPL])
                nc.gpsimd.tensor_mul(et[:, SPL:HW],
                                     ews[w][:, SPL:HW], ehs[b][:, SPL:HW])
                for blk in range(NBLK):
                    nc.tensor.matmul(outp[blk][:],
                                     v2[:, c * 256 + h * 128: c * 256 + (h + 1) * 128],
                                     et[:, blk * 512:(blk + 1) * 512],
                                     start=(k == 0), stop=(k == NCHUNK - 1))
            numh = nmp.tile([128, HW], BF16, tag="nm", name=f"numh{h}")
            if h == 0:
                # all copies on ACT: DVE must flow straight into head 1's
                # multiplies (ACT has plenty of slack here)
                for r in range(3):
                    a, b_ = 2 * r, 2 * r + 1
                    nc.scalar.copy(numh[:, a * 512:(a + 1) * 512], outp[a][:])
                    nc.scalar.copy(numh[:, b_ * 512:(b_ + 1) * 512], outp[b_][:])
                    nc.sync.dma_start(num_d[h][:, a * 512:(b_ + 1) * 512],
                                      numh[:, a * 512:(b_ + 1) * 512])
            else:
                # tail: ACT+DVE copy in parallel rounds, ship pairs as they
                # land
                for r in range(3):
                    a, b_ = 2 * r, 2 * r + 1
                    nc.scalar.copy(numh[:, a * 512:(a + 1) * 512], outp[a][:])
                    nc.vector.tensor_copy(numh[:, b_ * 512:(b_ + 1) * 512],
                                          outp[b_][:])
                    nc.sync.dma_start(num_d[h][:, a * 512:(b_ + 1) * 512],
                                      numh[:, a * 512:(b_ + 1) * 512])

        # ---- schedule ----
        prep_head(0)
        rep0 = rep_tiles(0)
        # V2 for both heads; fmapb columns are pre-permuted so natural
        # 128-col blocks are the blocked key chunks.
        for c in range(NCHUNK):
            vp = ps.tile([128, 256], F32, tag="ps", name=f"vp{c}")
            nc.tensor.matmul(vp[:], fmapb[:, c * 128:(c + 1) * 128], wvt[:],
                             start=True, stop=True)
            nc.scalar.copy(v2[:, c * 256:(c + 1) * 256], vp[:])
        prep_head(1)
        rep1 = rep_tiles(1)
        nc.sync.dma_start(eh_d[0][:], ehth[0][:])
        nc.sync.dma_start(ew_d[0][:], ewth[0][:])
        chunks_head(0, *rep0, STREAM)
        nc.sync.dma_start(eh_d[1][:], ehth[1][:])
        nc.sync.dma_start(ew_d[1][:], ewth[1][:])
        chunks_head(1, *rep1, STREAM)

    nc.compile()
    _cached["nc"] = nc
    return nc


def _prep_core_inputs(fmap_cb, Wqk, Wv, rel_h, rel_w, pair, perm):
    """Host-side input prep for one core. fmap_cb: (128, HW) f32 slice."""
    bf = ml_dtypes.bfloat16
    hg0 = pair * 2  # global head index of local head 0
    wvt = np.empty((128, 256), np.float32)
    het2 = np.empty((128, 2 * H * H), np.float32)
    wet2 = np.empty((128, 2 * W * W), np.float32)
    idx_h = np.arange(H)[:, None] - np.arange(H)[None, :] + (MAX_POS - 1)
    idx_w = np.arange(W)[:, None] - np.arange(W)[None, :] + (MAX_POS - 1)
    het = rel_h[idx_h].transpose(2, 0, 1).reshape(128, H * H)  # (d, x*48+u)
    wet = rel_w[idx_w].transpose(2, 0, 1).reshape(128, W * W)  # (d, y*64+v)
    for hl in range(2):
        hg = hg0 + hl
        wq = Wqk[hg * 128:(hg + 1) * 128, :]          # (d, c)
        wvt[:, hl * 128:(hl + 1) * 128] = Wv[hg * 128:(hg + 1) * 128, :].T
        het2[:, hl * H * H:(hl + 1) * H * H] = SCALE * (wq.T @ het)
        wet2[:, hl * W * W:(hl + 1) * W * W] = SCALE * (wq.T @ wet)
    fperm = np.empty_like(fmap_cb)
    fperm[:, perm] = fmap_cb
    return {
        "fmapb": fperm.astype(ml_dtypes.float8_e4m3fn),
        "wvt": wvt.astype(bf),
        "het2": het2.astype(ml_dtypes.float8_e4m3fn),
        "wet2": wet2.astype(ml_dtypes.float8_e4m3fn),
    }


def kernel(fmap1, fmap2, Wqk, Wv, rel_h, rel_w, Wp, gamma):
    fmap1 = np.asarray(fmap1, np.float32)
    fmap2 = np.asarray(fmap2, np.float32)
    Wqk = np.asarray(Wqk, np.float32)
    Wv = np.asarray(Wv, np.float32)
    rel_h = np.asarray(rel_h, np.float32)
    rel_w = np.asarray(rel_w, np.float32)
    Wp = np.asarray(Wp, np.float32)
    g = float(np.asarray(gamma).reshape(-1)[0])
    perm = _perm()  # perm[x*64+y] = device column of spatial (x,y)

    nc = _build_nc()
    fmaps = [fmap1, fmap2]
    in_maps = []
    core_meta = []
    for pair in range(2):
        for f in range(2):
            for b in range(B):
                fm = fmaps[f][b].reshape(DIM, HW)
                in_maps.append(_prep_core_inputs(fm, Wqk, Wv, rel_h, rel_w,
                                                 pair, perm))
                core_meta.append((pair, f, b))

    res = bass_utils.run_bass_kernel_spmd(nc, in_maps, core_ids=list(range(8)))

    outs = [np.array(fmaps[f], np.float32).copy() for f in range(2)]
    for core, (pair, f, b) in enumerate(core_meta):
        r = res.results[core]
        for hl in range(2):
            hg = pair * 2 + hl
            num = np.asarray(r[f"num{hl}"], np.float32)       # (128, HW) permuted
            den = (np.asarray(r[f"eh{hl}"], np.float32).sum(0)
                   * np.asarray(r[f"ew{hl}"], np.float32).sum(0))  # permuted
            attn = num / den[None, :]
            attn = attn[:, perm]                              # back to spatial
            proj = g * (Wp[:, hg * 128:(hg + 1) * 128] @ attn)
            outs[f][b] += proj.reshape(DIM, H, W)
    return outs[0], outs[1]
